# revision 39
# baseline (speedup 1.0000x reference)
"""Trainium2 Bass kernel for nn_HermesMessageLayer (gnn_message_passing).

Math: out[e,i,n] = sum_{b,f,r,j,m} inp[e,j,m] * precomp[e,f,r]
                                   * kernel[b,f,n,m] * weight[b,r,i,j] + bias[i]

Staging (per core, data-parallel over E across 8 cores):
  KW[(j,m), (i,n,f,r)] = sum_b kernel[b,f,n,m]*weight[b,r,i,j]   (host, tiny)
  t[e, (ni, fr)] = inp[e,(j,m)] @ KW                             (PE matmul)
  out[e, ni]     = sum_fr pc[e,fr] * t[e, ni, fr]                (DVE, fr innermost)

Key layout choice: fr is the INNERMOST kw column index, so the per-edge
pc contraction vectorizes as ONE DVE tensor_tensor multiply per tile group
(pc broadcast along ni via a stride-0 AP dim; fr stride-1 innermost keeps
the DVE 2x 16-bit mode) followed by a short add tree, instead of 10
chained scalar_tensor_tensor ops per tile (no fast mode, serial stalls).

Pipeline per 128-edge tile pair:
  - inp is host-transposed to [jm, E] so group loads are plain contiguous
    DMAs (no XBAR transpose, no 96->128 row pad).
  - 4 PE matmuls (480 cols each) fill a 4-bank PSUM macro tile.
  - Scalar engine copies both tiles' t' PSUM->SBUF (fp16 cast) in one op.
Per tile group (16 tiles; short tail group):
  - DVE: prod = t' * pc  (one 2x-mode op), tree: 5+5 -> 2+2 -> 1+1 (+carry).
  - Vector engine is the bottleneck (~100% busy); Pool stays idle on
    purpose — its TT ops are ~8x slower and share SBUF ports with DVE.
  - fp16 HWDGE store; host un-permutes, upcasts, adds bias.
"""

import sys

import numpy as np

sys.path.insert(0, "/opt/trn_rl_repo")

import concourse.bass as bass
import concourse.bacc as bacc
import concourse.tile as tile
from concourse import mybir
from concourse.bass_utils import run_bass_kernel_spmd

# Problem dims
E, J, I = 300000, 32, 32
M, N = 3, 3
B, F, R = 6, 5, 2
JM = J * M          # 96
NI = I * N          # 96  (ni = i*3 + n)
FR = F * R          # 10  (fr = f*2 + r)
TCOLS = NI * FR     # 960 (col = ni*10 + fr; fr innermost)

NCORES = 8
E_CORE = E // NCORES            # 37500
G = 16                          # tiles per full group
TILE_E = 128                    # edges per tile (PSUM partitions)
GROUP_E = G * TILE_E            # 2048
NG = -(-E_CORE // GROUP_E)      # 19 groups
G_LAST = -(-(E_CORE - (NG - 1) * GROUP_E) // TILE_E)  # 5 tiles in tail group
NT = (NG - 1) * G + G_LAST      # 293 tiles
E_PAD = NT * TILE_E             # 37504

F16 = mybir.dt.float16
F32 = mybir.dt.float32

_mult = mybir.AluOpType.mult
_add = mybir.AluOpType.add


def _group_schedule():
    """(tile_offset, n_tiles) per chunk: staircase start (2,3,6,10,12 tiles)
    so the DVE pipeline fills early, then full 16s, 4-tile tail."""
    sizes = [1, 2, 3, 6, 10, 12] + [16] * 16 + [3]
    sched, toff = [], 0
    for s in sizes:
        sched.append((toff, s))
        toff += s
    assert toff == NT
    return sched


def build_program():
    """Build the single-core Bass program (same program runs SPMD on all cores)."""
    nc = bacc.Bacc("TRN2", target_bir_lowering=False, debug=False)

    inpT_t = nc.dram_tensor("inpT", [JM, E_PAD], F16, kind="ExternalInput").ap()
    # pc halves duplicated (cols = [p0..p4 p0..p4 p5..p9 p5..p9]) so each
    # split multiply's broadcast operand has 20-byte innermost runs
    pc_t = nc.dram_tensor("pc", [128, NT, 2 * FR], F16, kind="ExternalInput").ap()
    kw_t = nc.dram_tensor("kw", [JM, TCOLS], F16, kind="ExternalInput").ap()
    out_t = nc.dram_tensor("out", [128, NT, NI], F16, kind="ExternalOutput").ap()

    with tile.TileContext(nc) as tc:
        with (
            tc.tile_pool(name="const", bufs=1) as const_pool,
            tc.tile_pool(name="inpT", bufs=2) as inpT_pool,
            tc.tile_pool(name="pc", bufs=2) as pc_pool,
            tc.tile_pool(name="tsb", bufs=3) as tsb_pool,
            tc.tile_pool(name="prodA", bufs=1) as prodA_pool,
            tc.tile_pool(name="prodB", bufs=1) as prodB_pool,
            tc.tile_pool(name="r1", bufs=1) as r1_pool,
            tc.tile_pool(name="r2", bufs=1) as r2_pool,
            tc.tile_pool(name="s1", bufs=1) as s1_pool,
            tc.tile_pool(name="acc", bufs=3) as acc_pool,
            tc.tile_pool(name="psum", bufs=2, space="PSUM") as psum_pool,
        ):
            kw_sb = const_pool.tile([JM, TCOLS], F16)
            nc.sync.dma_start(kw_sb[:], kw_t[:])

            # PE pstate warmup: dummy matmuls on a zeroed tile run during the
            # initial DMA window, so the first real matmuls start at speed
            # (cold PE runs at 0.65-1.2 GHz until ~3us of continuous work).
            warm = const_pool.tile([JM, 480], F16)
            nc.gpsimd.memset(warm[:], 0)
            warmed = False

            for toff, gg in _group_schedule():
                ge = gg * TILE_E
                inpT = inpT_pool.tile([JM, ge], F16)
                nc.sync.dma_start(
                    inpT[:], inpT_t[:, toff * TILE_E : toff * TILE_E + ge]
                )
                pc = pc_pool.tile([128, gg, 2 * FR], F16)
                nc.sync.dma_start(pc[:], pc_t[:, toff : toff + gg])

                tsb = tsb_pool.tile([128, gg, TCOLS], F16)
                for gp in range(-(-gg // 2)):
                    nhalf = min(2, gg - gp * 2)
                    # macro PSUM tile: 2 edge-tiles x 2 banks each
                    ps = psum_pool.tile([128, 2048], F32)
                    if not warmed:
                        # dummy matmuls depend only on the memset, so they run
                        # during the initial DMA window and ramp the PE pstate
                        warmed = True
                        for _ in range(2):
                            nc.tensor.matmul(
                                ps[:, 0:480], warm[:, 0:128], warm[:],
                                start=True, stop=True,
                            )
                    for half in range(nhalf):
                        gi = gp * 2 + half
                        lhsT = inpT[:, gi * TILE_E : (gi + 1) * TILE_E]
                        base = half * 1024
                        nc.tensor.matmul(
                            ps[:, base : base + 480],
                            lhsT,
                            kw_sb[:, 0:480],
                            start=True,
                            stop=True,
                        )
                        nc.tensor.matmul(
                            ps[:, base + 512 : base + 992],
                            lhsT,
                            kw_sb[:, 480:960],
                            start=True,
                            stop=True,
                        )
                    # one ACT copy drains the macro tile (f32 -> f16 cast)
                    ps_view = ps[:, 0 : nhalf * 1024].rearrange(
                        "p (b x) -> p b x", b=2 * nhalf
                    )[:, :, 0:480]
                    tsb_view = tsb[:, gp * 2 : gp * 2 + nhalf].rearrange(
                        "p g (b x) -> p (g b) x", b=2
                    )
                    nc.scalar.copy(tsb_view, ps_view)

                # Split pc-multiply: tsb cols are half-major [g, half, NI, 5]
                # (kw packs col = half*480 + ni*5 + k). Each mult reads a
                # contiguous 480-col block viewed [48, 10] (two ni per row),
                # with the duplicated-pc broadcast operand giving 20B runs —
                # both at the clean DVE 2x rate. A/B land in separate tiles
                # so every tree add reads two distinct SBUF regions.
                tsb5 = tsb[:].rearrange("p g (h x k) -> p g h x k", h=2, k=FR)
                prodA = prodA_pool.tile([128, gg, NI // 2, FR], F16)
                prodB = prodB_pool.tile([128, gg, NI // 2, FR], F16)
                pcbA = pc[:, :, None, 0:FR].to_broadcast([128, gg, NI // 2, FR])
                pcbB = pc[:, :, None, FR : 2 * FR].to_broadcast(
                    [128, gg, NI // 2, FR]
                )
                nc.vector.tensor_tensor(prodA[:], tsb5[:, :, 0], pcbA, _mult)
                nc.vector.tensor_tensor(prodB[:], tsb5[:, :, 1], pcbB, _mult)

                # fr-sum tree: (5+5) -> 5 -> (2,2,1) -> 1; r1[g,ni,k] layout
                r1 = r1_pool.tile([128, gg, NI, 5], F16)
                nc.vector.tensor_tensor(r1[:], prodA[:], prodB[:], _add)
                # r2 written pair-major: r2 itself drops to 1x (permuted out
                # AP), but both s1 operands become contiguous 2x reads — a
                # net win (1536 elems gain 2x vs 3072 losing ~20%).
                r2 = r2_pool.tile([128, gg, 2, NI], F16)
                r2v = r2[:].rearrange("p g two ni -> p g ni two")
                nc.vector.tensor_tensor(
                    r2v, r1[:, :, :, 0:2], r1[:, :, :, 2:4], _add
                )
                s1 = s1_pool.tile([128, gg, NI], F16)
                nc.vector.tensor_tensor(s1[:], r2[:, :, 0], r2[:, :, 1], _add)
                acc = acc_pool.tile([128, gg, NI], F16)
                nc.vector.tensor_tensor(acc[:], s1[:], r1[:, :, :, 4], _add)

                nc.sync.dma_start(out_t[:, toff : toff + gg], acc[:])

    nc.compile()
    return nc


def _pack_core(inp_c, precomp_c):
    """Pack one core's slice into the padded/permuted device layouts."""
    e_c = inp_c.shape[0]
    inpT = np.zeros([JM, E_PAD], dtype=np.float16)
    inpT[:, :e_c] = inp_c.reshape(e_c, JM).astype(np.float16).T

    pc_pad = np.zeros([E_PAD, FR], dtype=np.float16)
    pc_pad[:e_c] = precomp_c.reshape(e_c, FR).astype(np.float16)
    # tile t partition p holds edge t*TILE_E + p; halves duplicated:
    # cols = [p0..p4 p0..p4 p5..p9 p5..p9] (20B-run broadcast operands)
    pc_perm = np.ascontiguousarray(
        pc_pad.reshape(NT, TILE_E, FR).transpose(1, 0, 2)
    )
    pc_dup = np.concatenate(
        [pc_perm[:, :, 0:5], pc_perm[:, :, 0:5],
         pc_perm[:, :, 5:10], pc_perm[:, :, 5:10]],
        axis=2,
    )
    return inpT, np.ascontiguousarray(pc_dup)


def _pack_shared(kernel, weight, bias=None):
    # KW[(j,m), (i,n,f,r)] = sum_b kernel[b,f,n,m] * weight[b,r,i,j]
    # half-major columns: col = (fr//5)*480 + ni*5 + fr%5, so each split
    # pc-multiply reads one contiguous 480-col block.
    kw = np.einsum(
        "bfnm,brij->jminfr",
        kernel.astype(np.float64),
        weight.astype(np.float64),
    ).reshape(JM, NI, 2, 5)
    kw = np.ascontiguousarray(kw.transpose(0, 2, 1, 3)).reshape(JM, TCOLS)
    return kw.astype(np.float16)


_PROGRAM_CACHE = {}


def _get_program():
    if "p" not in _PROGRAM_CACHE:
        _PROGRAM_CACHE["p"] = build_program()
    return _PROGRAM_CACHE["p"]


def kernel(inp, precomp, kernel, weight, bias):
    inp = np.asarray(inp)
    precomp = np.asarray(precomp)
    kernel_np = np.asarray(kernel)
    weight = np.asarray(weight)
    bias = np.asarray(bias)

    kw_h = _pack_shared(kernel_np, weight)

    in_maps = []
    for c in range(NCORES):
        sl = slice(c * E_CORE, (c + 1) * E_CORE)
        inpT, pc_perm = _pack_core(inp[sl], precomp[sl])
        in_maps.append({"inpT": inpT, "pc": pc_perm, "kw": kw_h})

    nc = _get_program()
    res = run_bass_kernel_spmd(nc, in_maps, list(range(NCORES)))

    out = np.empty([E, I, N], dtype=np.float32)
    for c in range(NCORES):
        o = np.asarray(res.results[c]["out"]).astype(np.float32)  # [128,NT,NI]
        o = o.transpose(1, 0, 2).reshape(E_PAD, NI)[:E_CORE]
        out[c * E_CORE : (c + 1) * E_CORE] = o.reshape(E_CORE, I, N)
    if bias.any():
        out += bias.astype(np.float32)[None, :, None]
    return out


# revision 41
# speedup vs baseline: 1.5176x; 1.5176x over previous
"""Trainium2 Bass kernel for nn_HermesMessageLayer (gnn_message_passing).

Math: out[e,i,n] = sum_{b,f,r,j,m} inp[e,j,m] * precomp[e,f,r]
                                   * kernel[b,f,n,m] * weight[b,r,i,j] + bias[i]

Staging (per core, data-parallel over E across 8 cores):
  KW[(j,m), (i,n,f,r)] = sum_b kernel[b,f,n,m]*weight[b,r,i,j]   (host, tiny)
  t[e, (ni, fr)] = inp[e,(j,m)] @ KW                             (PE matmul)
  out[e, ni]     = sum_fr pc[e,fr] * t[e, ni, fr]                (DVE, fr innermost)

Key layout choice: fr is the INNERMOST kw column index, so the per-edge
pc contraction vectorizes as ONE DVE tensor_tensor multiply per tile group
(pc broadcast along ni via a stride-0 AP dim; fr stride-1 innermost keeps
the DVE 2x 16-bit mode) followed by a short add tree, instead of 10
chained scalar_tensor_tensor ops per tile (no fast mode, serial stalls).

Pipeline per 128-edge tile pair:
  - inp is host-transposed to [jm, E] so group loads are plain contiguous
    DMAs (no XBAR transpose, no 96->128 row pad).
  - 4 PE matmuls (480 cols each) fill a 4-bank PSUM macro tile.
  - Scalar engine copies both tiles' t' PSUM->SBUF (fp16 cast) in one op.
Per tile group (16 tiles; short tail group):
  - DVE: prod = t' * pc  (one 2x-mode op), tree: 5+5 -> 2+2 -> 1+1 (+carry).
  - Vector engine is the bottleneck (~100% busy); Pool stays idle on
    purpose — its TT ops are ~8x slower and share SBUF ports with DVE.
  - fp16 HWDGE store; host un-permutes, upcasts, adds bias.
"""

import sys

import numpy as np

sys.path.insert(0, "/opt/trn_rl_repo")

import concourse.bass as bass
import concourse.bacc as bacc
import concourse.tile as tile
from concourse import mybir
from concourse.bass_utils import run_bass_kernel_spmd

# Problem dims
E, J, I = 300000, 32, 32
M, N = 3, 3
B, F, R = 6, 5, 2
JM = J * M          # 96
NI = I * N          # 96  (ni = i*3 + n)
FR = F * R          # 10  (fr = f*2 + r)
TCOLS = NI * FR     # 960 (col = ni*10 + fr; fr innermost)

NCORES = 8
E_CORE = E // NCORES            # 37500
G = 16                          # tiles per full group
TILE_E = 128                    # edges per tile (PSUM partitions)
GROUP_E = G * TILE_E            # 2048
NG = -(-E_CORE // GROUP_E)      # 19 groups
G_LAST = -(-(E_CORE - (NG - 1) * GROUP_E) // TILE_E)  # 5 tiles in tail group
NT = (NG - 1) * G + G_LAST      # 293 tiles
E_PAD = NT * TILE_E             # 37504

F16 = mybir.dt.float16
F32 = mybir.dt.float32

_mult = mybir.AluOpType.mult
_add = mybir.AluOpType.add


def _group_schedule():
    """(tile_offset, n_tiles) per chunk: staircase start (2,3,6,10,12 tiles)
    so the DVE pipeline fills early, then full 16s, 4-tile tail."""
    sizes = [1, 2, 3, 6, 10, 12] + [16] * 16 + [3]
    sched, toff = [], 0
    for s in sizes:
        sched.append((toff, s))
        toff += s
    assert toff == NT
    return sched


def build_program():
    """Build the single-core Bass program (same program runs SPMD on all cores)."""
    nc = bacc.Bacc("TRN2", target_bir_lowering=False, debug=False)

    inpT_t = nc.dram_tensor("inpT", [JM, E_PAD], F16, kind="ExternalInput").ap()
    # pc halves duplicated (cols = [p0..p4 p0..p4 p5..p9 p5..p9]) so each
    # split multiply's broadcast operand has 20-byte innermost runs
    pc_t = nc.dram_tensor("pc", [128, NT, 2 * FR], F16, kind="ExternalInput").ap()
    kw_t = nc.dram_tensor("kw", [JM, TCOLS], F16, kind="ExternalInput").ap()
    out_t = nc.dram_tensor("out", [128, NT, NI], F16, kind="ExternalOutput").ap()

    with tile.TileContext(nc) as tc:
        with (
            tc.tile_pool(name="const", bufs=1) as const_pool,
            tc.tile_pool(name="inpT", bufs=2) as inpT_pool,
            tc.tile_pool(name="pc", bufs=2) as pc_pool,
            tc.tile_pool(name="tsb", bufs=3) as tsb_pool,
            tc.tile_pool(name="prodA", bufs=1) as prodA_pool,
            tc.tile_pool(name="prodB", bufs=1) as prodB_pool,
            tc.tile_pool(name="r1", bufs=1) as r1_pool,
            tc.tile_pool(name="r2", bufs=1) as r2_pool,
            tc.tile_pool(name="s1", bufs=1) as s1_pool,
            tc.tile_pool(name="acc", bufs=3) as acc_pool,
            tc.tile_pool(name="psum", bufs=2, space="PSUM") as psum_pool,
        ):
            kw_sb = const_pool.tile([JM, TCOLS], F16)
            nc.sync.dma_start(kw_sb[:], kw_t[:])

            # PE pstate warmup: dummy matmuls on a zeroed tile run during the
            # initial DMA window, so the first real matmuls start at speed
            # (cold PE runs at 0.65-1.2 GHz until ~3us of continuous work).
            warm = const_pool.tile([JM, 480], F16)
            nc.gpsimd.memset(warm[:], 0)
            warmed = False

            for toff, gg in _group_schedule():
                ge = gg * TILE_E
                inpT = inpT_pool.tile([JM, ge], F16)
                nc.sync.dma_start(
                    inpT[:], inpT_t[:, toff * TILE_E : toff * TILE_E + ge]
                )
                pc = pc_pool.tile([128, gg, 2 * FR], F16)
                # pc loads ride the idle GPSIMD's software DGE so the Sync
                # queue's ~600ns/descriptor issue slots go to inpT/out DMAs
                nc.gpsimd.dma_start(out=pc[:], in_=pc_t[:, toff : toff + gg])

                tsb = tsb_pool.tile([128, gg, TCOLS], F16)
                for gp in range(-(-gg // 2)):
                    nhalf = min(2, gg - gp * 2)
                    # macro PSUM tile: 2 edge-tiles x 2 banks each
                    ps = psum_pool.tile([128, 2048], F32)
                    if not warmed:
                        # dummy matmuls depend only on the memset, so they run
                        # during the initial DMA window and ramp the PE pstate
                        warmed = True
                        for _ in range(2):
                            nc.tensor.matmul(
                                ps[:, 0:480], warm[:, 0:128], warm[:],
                                start=True, stop=True,
                            )
                    for half in range(nhalf):
                        gi = gp * 2 + half
                        lhsT = inpT[:, gi * TILE_E : (gi + 1) * TILE_E]
                        base = half * 1024
                        nc.tensor.matmul(
                            ps[:, base : base + 480],
                            lhsT,
                            kw_sb[:, 0:480],
                            start=True,
                            stop=True,
                        )
                        nc.tensor.matmul(
                            ps[:, base + 512 : base + 992],
                            lhsT,
                            kw_sb[:, 480:960],
                            start=True,
                            stop=True,
                        )
                    # one ACT copy drains the macro tile (f32 -> f16 cast)
                    ps_view = ps[:, 0 : nhalf * 1024].rearrange(
                        "p (b x) -> p b x", b=2 * nhalf
                    )[:, :, 0:480]
                    tsb_view = tsb[:, gp * 2 : gp * 2 + nhalf].rearrange(
                        "p g (b x) -> p (g b) x", b=2
                    )
                    nc.scalar.copy(tsb_view, ps_view)

                # Split pc-multiply: tsb cols are half-major [g, half, NI, 5]
                # (kw packs col = half*480 + ni*5 + k). Each mult reads a
                # contiguous 480-col block viewed [48, 10] (two ni per row),
                # with the duplicated-pc broadcast operand giving 20B runs —
                # both at the clean DVE 2x rate. A/B land in separate tiles
                # so every tree add reads two distinct SBUF regions.
                tsb5 = tsb[:].rearrange("p g (h x k) -> p g h x k", h=2, k=FR)
                prodA = prodA_pool.tile([128, gg, NI // 2, FR], F16)
                prodB = prodB_pool.tile([128, gg, NI // 2, FR], F16)
                pcbA = pc[:, :, None, 0:FR].to_broadcast([128, gg, NI // 2, FR])
                pcbB = pc[:, :, None, FR : 2 * FR].to_broadcast(
                    [128, gg, NI // 2, FR]
                )
                nc.vector.tensor_tensor(prodA[:], tsb5[:, :, 0], pcbA, _mult)
                nc.vector.tensor_tensor(prodB[:], tsb5[:, :, 1], pcbB, _mult)

                # fr-sum tree: (5+5) -> 5 -> (2,2,1) -> 1; r1[g,ni,k] layout
                r1 = r1_pool.tile([128, gg, NI, 5], F16)
                nc.vector.tensor_tensor(r1[:], prodA[:], prodB[:], _add)
                r2 = r2_pool.tile([128, gg, NI, 2], F16)
                nc.vector.tensor_tensor(
                    r2[:], r1[:, :, :, 0:2], r1[:, :, :, 2:4], _add
                )
                s1 = s1_pool.tile([128, gg, NI], F16)
                nc.vector.tensor_tensor(s1[:], r2[:, :, :, 0], r2[:, :, :, 1], _add)
                acc = acc_pool.tile([128, gg, NI], F16)
                nc.vector.tensor_tensor(acc[:], s1[:], r1[:, :, :, 4], _add)

                nc.sync.dma_start(out_t[:, toff : toff + gg], acc[:])

    nc.compile()
    return nc


def _pack_core(inp_c, precomp_c):
    """Pack one core's slice into the padded/permuted device layouts."""
    e_c = inp_c.shape[0]
    inpT = np.zeros([JM, E_PAD], dtype=np.float16)
    inpT[:, :e_c] = inp_c.reshape(e_c, JM).astype(np.float16).T

    pc_pad = np.zeros([E_PAD, FR], dtype=np.float16)
    pc_pad[:e_c] = precomp_c.reshape(e_c, FR).astype(np.float16)
    # tile t partition p holds edge t*TILE_E + p; halves duplicated:
    # cols = [p0..p4 p0..p4 p5..p9 p5..p9] (20B-run broadcast operands)
    pc_perm = np.ascontiguousarray(
        pc_pad.reshape(NT, TILE_E, FR).transpose(1, 0, 2)
    )
    pc_dup = np.concatenate(
        [pc_perm[:, :, 0:5], pc_perm[:, :, 0:5],
         pc_perm[:, :, 5:10], pc_perm[:, :, 5:10]],
        axis=2,
    )
    return inpT, np.ascontiguousarray(pc_dup)


def _pack_shared(kernel, weight, bias=None):
    # KW[(j,m), (i,n,f,r)] = sum_b kernel[b,f,n,m] * weight[b,r,i,j]
    # half-major columns: col = (fr//5)*480 + ni*5 + fr%5, so each split
    # pc-multiply reads one contiguous 480-col block.
    kw = np.einsum(
        "bfnm,brij->jminfr",
        kernel.astype(np.float64),
        weight.astype(np.float64),
    ).reshape(JM, NI, 2, 5)
    kw = np.ascontiguousarray(kw.transpose(0, 2, 1, 3)).reshape(JM, TCOLS)
    return kw.astype(np.float16)


_PROGRAM_CACHE = {}


def _get_program():
    if "p" not in _PROGRAM_CACHE:
        _PROGRAM_CACHE["p"] = build_program()
    return _PROGRAM_CACHE["p"]


def kernel(inp, precomp, kernel, weight, bias):
    inp = np.asarray(inp)
    precomp = np.asarray(precomp)
    kernel_np = np.asarray(kernel)
    weight = np.asarray(weight)
    bias = np.asarray(bias)

    kw_h = _pack_shared(kernel_np, weight)

    in_maps = []
    for c in range(NCORES):
        sl = slice(c * E_CORE, (c + 1) * E_CORE)
        inpT, pc_perm = _pack_core(inp[sl], precomp[sl])
        in_maps.append({"inpT": inpT, "pc": pc_perm, "kw": kw_h})

    nc = _get_program()
    res = run_bass_kernel_spmd(nc, in_maps, list(range(NCORES)))

    out = np.empty([E, I, N], dtype=np.float32)
    for c in range(NCORES):
        o = np.asarray(res.results[c]["out"]).astype(np.float32)  # [128,NT,NI]
        o = o.transpose(1, 0, 2).reshape(E_PAD, NI)[:E_CORE]
        out[c * E_CORE : (c + 1) * E_CORE] = o.reshape(E_CORE, I, N)
    if bias.any():
        out += bias.astype(np.float32)[None, :, None]
    return out
